# revision 28
# baseline (speedup 1.0000x reference)
"""Trainium2 Bass kernel for CombinedLabelDistributionLoss.

Strategy (8 NeuronCores, SPMD, no collectives):
  - Batch-parallel: core c owns rows [8c, 8c+8) of predictions/targets.
  - Pearson is computed from RAW-data sums (sum x, y, xy, x^2, y^2);
    standardization cancels algebraically; finished on host.
  - The 140-bin DFT over the flattened standardized signal uses the
    angle-addition factorization  sin(theta*(256*o + r)) =
    sinO[o]cosI[r] + cosO[o]sinI[r].  The device DFTs the RAW windowed
    signal (x*hann), contracting over r first (so the row identity, which
    lives in o = n//256, survives on the partition axis), and emits
    per-o partial products; the per-row reduction and the per-row
    standardization correction
        sin_part = sum_b inv_b * (S_sin[b] - mu_b * W_sin[b])
    happen on host (W_* are host-precomputed window-only DFT partials).
  - DMA-completion semaphores serialize per issuing engine (~2.2us each),
    so inputs are merged into one or two DMAs per engine (SP / Activation
    / Pool) with the critical ones first, and there is exactly one output
    DMA on each of two engines.
  - bf16 feeds the PE and the combine multiplies (validated: final rel
    err ~7e-6); PSUM stays f32; Pearson stats stay f32.
"""

import math

import numpy as np

B, T = 64, 4096
NCORES = 8
RPC = B // NCORES          # rows per core = 8
P = 128                    # SBUF partitions
F = (RPC * T) // P         # free dim = 256
NBINS = 140
N = B * T                  # 262144

_built = None


def _build_module():
    import concourse.bacc as bacc
    import concourse.bass as bass
    import concourse.mybir as mybir
    from concourse import tile

    f32 = mybir.dt.float32
    bf16 = mybir.dt.bfloat16
    AT = mybir.ActivationFunctionType
    ALU = mybir.AluOpType
    AX = mybir.AxisListType

    nc = bacc.Bacc(target_bir_lowering=False)

    # xti = [xt_bf16 | wint | innert]: signal + window + PE inner tables
    xti_d = nc.dram_tensor("xti", [P, 2 * F + 4 * NBINS], bf16, kind="ExternalInput")
    outer_d = nc.dram_tensor("outer", [P, 4 * NBINS], bf16, kind="ExternalInput")
    xy_d = nc.dram_tensor("xyin", [P, 2 * F], f32, kind="ExternalInput")
    prods_d = nc.dram_tensor("prods", [P, 4 * NBINS], bf16, kind="ExternalOutput")
    stats_d = nc.dram_tensor("stats", [P, 5], f32, kind="ExternalOutput")

    with tile.TileContext(nc) as tc:
        with (
            tc.tile_pool(name="sb", bufs=1) as pool,
            tc.tile_pool(name="ps", bufs=1, space=bass.MemorySpace.PSUM) as psum,
        ):
            # ---- input DMAs: one or two per engine, critical first ----
            xti = pool.tile([P, 2 * F + 4 * NBINS], bf16)
            nc.sync.dma_start(xti[:], xti_d[:])
            outer = pool.tile([P, 4 * NBINS], bf16)
            nc.scalar.dma_start(outer[:], outer_d[:])
            xyin = pool.tile([P, 2 * F], f32)
            nc.gpsimd.dma_start(xyin[:], xy_d[:])
            x = xyin[:, 0:F]
            y = xyin[:, F:2 * F]
            xt = xti[:, 0:F]
            wint = xti[:, F:2 * F]
            innert = xti[:, 2 * F:2 * F + 4 * NBINS]

            # ---- DFT path (critical): raw windowed signal, per-o partials ----
            xw = pool.tile([P, F], bf16)
            nc.vector.tensor_mul(xw[:], xt, wint)
            U_ps = psum.tile([P, 2 * NBINS], f32)
            nc.tensor.matmul(U_ps[:], xw[:, 0:P], innert[:, 0:2 * NBINS],
                             start=True, stop=False)
            nc.tensor.matmul(U_ps[:], xw[:, P:2 * P], innert[:, 2 * NBINS:4 * NBINS],
                             start=False, stop=True)
            prods = pool.tile([P, 4 * NBINS], bf16)
            nc.vector.tensor_mul(prods[:, 0:2 * NBINS], U_ps[:],
                                 outer[:, 0:2 * NBINS])
            nc.vector.tensor_mul(prods[:, 2 * NBINS:4 * NBINS], U_ps[:],
                                 outer[:, 2 * NBINS:4 * NBINS])
            nc.sync.dma_start(prods_d[:], prods[:])

            # ---- Pearson raw stats: [sx, sy, sxy, sx2, sy2] ----
            stats = pool.tile([P, 5], f32)
            scr = pool.tile([P, F], f32)
            nc.scalar.activation(scr[:], x, AT.Square, accum_out=stats[:, 3:4])
            nc.scalar.activation(scr[:], y, AT.Square, accum_out=stats[:, 4:5])
            nc.vector.reduce_sum(out=stats[:, 0:1], in_=x, axis=AX.X, op=ALU.add)
            nc.vector.reduce_sum(out=stats[:, 1:2], in_=y, axis=AX.X, op=ALU.add)
            xy = pool.tile([P, F], f32)
            nc.gpsimd.tensor_mul(xy[:], x, y)
            nc.vector.reduce_sum(out=stats[:, 2:3], in_=xy[:], axis=AX.X, op=ALU.add)
            nc.gpsimd.dma_start(stats_d[:], stats[:])

    nc.compile()
    return nc


def _tables(frame_rate: int):
    """Host-precomputed constant tables (depend only on frame_rate)."""
    import ml_dtypes

    nbf = ml_dtypes.bfloat16
    bpm = np.arange(40.0, 180.0, dtype=np.float32)
    k32 = (bpm / np.float32(60.0)) / (np.float32(frame_rate) / np.float32(N))
    theta = k32.astype(np.float64) * (2.0 * math.pi) / N       # [140]

    ov = np.arange(NCORES * P, dtype=np.float64)               # o = n // 256
    sinO = np.sin(theta[None, :] * F * ov[:, None])            # [1024, 140] f64
    cosO = np.cos(theta[None, :] * F * ov[:, None])
    rv = np.arange(F, dtype=np.float64)                        # r = n % 256
    sinI = np.sin(theta[None, :] * rv[:, None])                # [256, 140] f64
    cosI = np.cos(theta[None, :] * rv[:, None])

    # PE rhs for the r-contraction: per r-half h, [sinI_h | cosI_h]  (bf16)
    innert = np.concatenate(
        [sinI[0:P], cosI[0:P], sinI[P:2 * P], cosI[P:2 * P]], axis=1
    ).astype(nbf)                                              # [128, 560]

    # combine tables, per core (o rows): [cosO | sinO | sinO | cosO]
    outer = []
    for c in range(NCORES):
        sl = slice(c * P, (c + 1) * P)
        outer.append(np.ascontiguousarray(np.concatenate(
            [cosO[sl], sinO[sl], sinO[sl], cosO[sl]], axis=1).astype(nbf)))

    # transposed-layout hann window: wint[p, h*128+o_local] = w[256*o + 128*h + p]
    win = np.hanning(N).astype(np.float32)
    win_t = win.reshape(NCORES, P, 2, P).transpose(0, 3, 2, 1).reshape(NCORES, P, F)
    win_t = win_t.astype(nbf)

    # per-row window-only DFT partials (f64) for the host-side correction
    win2 = win.reshape(NCORES * P, F).astype(np.float64)
    W_sin = np.zeros((B, NBINS)); W_cos = np.zeros((B, NBINS))
    for b in range(B):
        sl = slice(b * 16, (b + 1) * 16)
        A = win2[sl] @ cosI                                    # [16, 140]
        Bm = win2[sl] @ sinI
        W_sin[b] = (sinO[sl] * A + cosO[sl] * Bm).sum(0)
        W_cos[b] = (cosO[sl] * A - sinO[sl] * Bm).sum(0)

    return innert, outer, win_t, W_sin, W_cos


_tables_cache = {}


def _make_in_maps(preds, targs, frame_rate):
    import ml_dtypes

    nbf = ml_dtypes.bfloat16
    if frame_rate not in _tables_cache:
        _tables_cache[frame_rate] = _tables(frame_rate)
    innert, outer, win_t, _, _ = _tables_cache[frame_rate]
    in_maps = []
    for c in range(NCORES):
        xc = preds[c * RPC:(c + 1) * RPC].reshape(P, F)
        yc = targs[c * RPC:(c + 1) * RPC].reshape(P, F)
        # transposed layout: xt[p, h*128+o] = x_flat[256*o + 128*h + p]
        xtc = xc.reshape(P, 2, P).transpose(2, 1, 0).reshape(P, F).astype(nbf)
        in_maps.append({
            "xti": np.ascontiguousarray(
                np.concatenate([xtc, win_t[c], innert], axis=1)),
            "outer": outer[c],
            "xyin": np.ascontiguousarray(np.concatenate([xc, yc], axis=1)),
        })
    return in_maps


def _finish(results, avg_hr, a, b, frame_rate):
    _, _, _, W_sin, W_cos = _tables_cache[frame_rate]

    # ---- Pearson from raw per-partition sums: group 16 partitions -> row ----
    st = np.concatenate([results[c]["stats"] for c in range(NCORES)], axis=0)
    sums = st.astype(np.float32).reshape(B, P // RPC, 5).sum(axis=1)    # [64, 5]
    sum_x, sum_y, sum_xy, sum_x2, sum_y2 = (sums[:, i] for i in range(5))
    Nt = np.float32(T)
    pearson = (Nt * sum_xy - sum_x * sum_y) / np.sqrt(
        (Nt * sum_x2 - sum_x ** 2) * (Nt * sum_y2 - sum_y ** 2))
    loss_rppg = np.float32(np.mean(np.float32(1.0) - pearson, dtype=np.float32))

    # ---- spectrum: per-row raw partials + standardization correction ----
    pr = np.concatenate([results[c]["prods"] for c in range(NCORES)], axis=0)
    d = pr.astype(np.float64).reshape(B, P // RPC, 4 * NBINS).sum(axis=1)  # [64, 560]
    S_sin = d[:, 0:NBINS] + d[:, NBINS:2 * NBINS]
    S_cos = d[:, 3 * NBINS:4 * NBINS] - d[:, 2 * NBINS:3 * NBINS]
    mu = (sum_x / Nt).astype(np.float64)
    ssq = sum_x2.astype(np.float64) - sum_x.astype(np.float64) * mu
    inv = 1.0 / np.sqrt(ssq / (T - 1))
    sin_part = (inv[:, None] * (S_sin - mu[:, None] * W_sin)).sum(0)
    cos_part = (inv[:, None] * (S_cos - mu[:, None] * W_cos)).sum(0)
    sin_part = sin_part.astype(np.float32)
    cos_part = cos_part.astype(np.float32)

    ca = sin_part ** 2 + cos_part ** 2
    ca = (ca / np.sum(ca)).astype(np.float32)

    t_idx = avg_hr - 40
    i = np.arange(NBINS, dtype=np.float64)
    td = np.exp(-(i - t_idx) ** 2 / 2.0) / math.sqrt(2.0 * math.pi)
    td = np.maximum(td, 1e-15).astype(np.float32)

    m = np.max(ca)
    e = np.exp(ca - m)
    freq = (e / np.sum(e)).astype(np.float32)
    loss_kl = np.float32(np.sum(td * (np.log(td) - np.log(freq))) / np.float32(140.0))

    loss_ce = np.float32(np.log(np.sum(np.exp(ca - m))) + m - ca[t_idx])
    mae_hr = np.float32(abs(float(t_idx) - float(np.argmax(ca))))

    total = np.float32(a) * loss_rppg + np.float32(b) * (loss_ce + loss_kl)
    return (np.float32(total), np.float32(loss_rppg), np.float32(loss_kl),
            np.float32(loss_ce), np.float32(mae_hr))


def kernel(predictions, targets, avg_hr, frame_rate, a, b):
    from concourse.bass_utils import run_bass_kernel_spmd

    global _built
    if _built is None:
        _built = _build_module()

    preds = np.ascontiguousarray(predictions, dtype=np.float32)
    targs = np.ascontiguousarray(targets, dtype=np.float32)
    in_maps = _make_in_maps(preds, targs, int(frame_rate))
    res = run_bass_kernel_spmd(nc=_built, in_maps=in_maps,
                               core_ids=list(range(NCORES)))
    return _finish(res.results, int(avg_hr), int(a), int(b), int(frame_rate))
